# revision 72
# baseline (speedup 1.0000x reference)
"""Trainium2 Bass kernel for nn_CrossAttentionExpert.

Three single-query cross-attention "experts" (id/txt/vis), each attending over
the other two modalities (K=2 kv positions), outputs concatenated, fused by a
linear layer and LayerNorm.

Strategy: pure data parallel over 8 NeuronCores, batch 16384 -> 2048 rows/core.
Weights replicated.

Math per expert (K=2 softmax == sigmoid gate on the score difference):
  g   = sigmoid(q . (k_a - k_b) / sqrt(D))    per head
  o   = v_b + g * (v_a - v_b)
Since k_a - k_b = Wk (x_a - x_b) and v_a - v_b = Wv (x_a - x_b) (biases
cancel), each expert needs only 4 projections: Q = Wq x_q + bq,
Kd = Wk xd, Vd = Wv xd, Vb = Wv x_b + bv, with xd = x_a - x_b precomputed
host-side.  The per-expert output projection is folded into the fuse matmul:
  y = sum_i G_i o_i + b_y,  G_i = Wf[:, iE:(i+1)E] @ Wout_i  (host-side).

Host-side prep (numpy, outside NEFF): weight slices transposed + cast to bf16
in the exact DRAM layout the kernel streams; input features (and pair
differences) transposed to feature-major [E, BC] bf16 per core.

On-core dataflow: feature-major activations; weights are the stationary
operand (bf16, fp32 PSUM accumulation).  2048 rows/core in 4 passes of 512.
Per-head score reduction and gate broadcast use small constant selector
matmuls on the PE (partition-dim reductions/broadcasts).
"""

import sys

sys.path.insert(0, "/opt/trn_rl_repo")

import numpy as np

import concourse.bass as bass
import concourse.bacc as bacc
import concourse.mybir as mybir
import concourse.tile as tile
from concourse.masks import make_identity

F32 = mybir.dt.float32
BF16 = mybir.dt.bfloat16
FP16 = mybir.dt.float16
FP8 = mybir.dt.float8e4
W8SCALE = 64.0          # host-side scale on fp8 Wq/Wk (avoid e4m3 subnormals)
AF = mybir.ActivationFunctionType
ALU = mybir.AluOpType
AX = mybir.AxisListType

B, E, H, D = 16384, 1024, 16, 64
NCORES = 8
BC = B // NCORES          # 2048 rows per core
BSUB = 512                # rows per pass
NPASS = BC // BSUB        # 4
EC = E // 128             # 8 feature chunks
LN_EPS = 1e-5

EXPERTS = [  # (name, q_idx, kv_a_idx, kv_b_idx)
    ("id", 0, 1, 2),
    ("txt", 1, 0, 2),
    ("vis", 2, 0, 1),
]
# sharded inputs (stacked core-major on axis 0).  bf16: feats txt/vis (fuse
# H-path) + 3 pair diffs (Vd); fp8: per-expert q-feat + pair diff (score path).
SH_BF16 = ["xT_txt", "xT_vis", "xdT_id", "xdT_txt", "xdT_vis"]
SH_FP8 = ["xq8_id", "xq8_txt", "xq8_vis", "xd8_id", "xd8_txt", "xd8_vis"]
SH_FEATS = SH_BF16 + SH_FP8


def _build_selectors(nc, sel, selt):
    """sel: [128, 8*16] bf16, chunk k cols k*16:(k+1)*16:
         sel_k[d, h] = 1 iff h == 2k + d//64   (score head-reduce, lhsT)
       selt: [16, 8*128] bf16, chunk k cols k*128:(k+1)*128:
         selt_k[h, d] = 1 iff h == 2k + d//64  (gate head-broadcast, lhsT)
    Condition h == 2k + d//64  <=>  -63 <= 64h - 128k - d <= 0.
    """
    nc.gpsimd.memset(sel, 1.0)
    nc.gpsimd.memset(selt, 1.0)
    for k in range(8):
        s = sel[:, k * 16:(k + 1) * 16]
        nc.gpsimd.affine_select(out=s, in_=s, compare_op=ALU.is_ge, fill=0.0,
                                base=128 * k, pattern=[[-64, 16]],
                                channel_multiplier=1)
        nc.gpsimd.affine_select(out=s, in_=s, compare_op=ALU.is_ge, fill=0.0,
                                base=63 - 128 * k, pattern=[[64, 16]],
                                channel_multiplier=-1)
        t = selt[:, k * 128:(k + 1) * 128]
        nc.gpsimd.affine_select(out=t, in_=t, compare_op=ALU.is_ge, fill=0.0,
                                base=128 * k, pattern=[[1, 128]],
                                channel_multiplier=-64)
        nc.gpsimd.affine_select(out=t, in_=t, compare_op=ALU.is_ge, fill=0.0,
                                base=63 - 128 * k, pattern=[[-1, 128]],
                                channel_multiplier=64)


def _mm(nc, out, lhsT, rhs, start, stop):
    nc.tensor.matmul(out, lhsT=lhsT, rhs=rhs, start=start, stop=stop)


def build_program(iters=1, passes=NPASS):
    nc = bacc.Bacc("TRN2", target_bir_lowering=False, debug=False,
                   num_devices=NCORES)

    xT_aps = {n: nc.dram_tensor(n, [E, BC], BF16, kind="ExternalInput").ap()
              for n in SH_BF16}
    xT_aps.update({n: nc.dram_tensor(n, [E, BC], FP8,
                                     kind="ExternalInput").ap()
                   for n in SH_FP8})
    W = {}
    for name, _, _, _ in EXPERTS:
        for kind in ("wv", "g"):
            W[f"{kind}_{name}"] = nc.dram_tensor(
                f"{kind}T_{name}", [E, E], BF16, kind="ExternalInput").ap()
        for kind in ("wq8", "wk8"):
            W[f"{kind}_{name}"] = nc.dram_tensor(
                f"{kind}T_{name}", [E, E], FP8, kind="ExternalInput").ap()
    # combined H matrices: experts id+txt share x_b = vis; vis uses x_b = txt
    W["h_idtxt"] = nc.dram_tensor("hT_idtxt", [E, E], BF16,
                                  kind="ExternalInput").ap()
    W["h_vis"] = nc.dram_tensor("hT_vis", [E, E], BF16,
                                kind="ExternalInput").ap()
    b_in = {n: nc.dram_tensor(f"b_in_{n}", [3 * E], F32,
                              kind="ExternalInput").ap()
            for n, _, _, _ in EXPERTS}
    b_y = nc.dram_tensor("b_y", [E], F32, kind="ExternalInput").ap()
    ln_g = nc.dram_tensor("ln_g", [E], F32, kind="ExternalInput").ap()
    ln_b = nc.dram_tensor("ln_b", [E], F32, kind="ExternalInput").ap()
    out_ap = nc.dram_tensor("outT", [E, BC], F32,
                            kind="ExternalOutput").ap()

    with tile.TileContext(nc) as tc:
        _emit(nc, tc, xT_aps, W, b_in, b_y, ln_g, ln_b, out_ap, iters, passes)
    nc.compile()
    return nc


def _emit(nc, tc, xT_aps, W, b_in, b_y, ln_g, ln_b, out_ap,
          iters=1, passes=NPASS):
    from contextlib import ExitStack
    ctx = ExitStack()
    with ctx:
        # ---------------- pools ----------------
        consts = ctx.enter_context(tc.tile_pool(name="consts", bufs=1))
        mm_ps = ctx.enter_context(tc.tile_pool(name="mm_ps", bufs=4, space="PSUM"))
        at_ps = ctx.enter_context(tc.tile_pool(name="at_ps", bufs=3, space="PSUM"))
        st_ps = ctx.enter_context(tc.tile_pool(name="st_ps", bufs=1, space="PSUM"))

        # ---------------- constants ----------------
        ones_col = consts.tile([128, 1], FP16, tag="ones_col")
        nc.vector.memset(ones_col, 1.0)
        ones_row = consts.tile([1, 128], FP16, tag="ones_row")
        nc.vector.memset(ones_row, 1.0)
        sel = consts.tile([128, 8 * 16], BF16, tag="sel")
        selt = consts.tile([16, 8 * 128], BF16, tag="selt")
        _build_selectors(nc, sel, selt)

        bias_in_sb = {}
        for name, _, _, _ in EXPERTS:
            t = consts.tile([128, 24], F32, tag=f"bin_{name}")
            nc.gpsimd.dma_start(t, b_in[name].rearrange("(c p) -> p c", p=128))
            bias_in_sb[name] = t
        by_sb = consts.tile([128, 8], F32, tag="b_y")
        nc.gpsimd.dma_start(by_sb, b_y.rearrange("(c p) -> p c", p=128))

        g_sb = consts.tile([128, 8], F32, tag="ln_g")
        nc.gpsimd.dma_start(g_sb, ln_g.rearrange("(c p) -> p c", p=128))
        lb_sb = consts.tile([128, 8], F32, tag="ln_b")
        nc.gpsimd.dma_start(lb_sb, ln_b.rearrange("(c p) -> p c", p=128))
        eps_sb = consts.tile([128, 1], F32, tag="eps")
        nc.vector.memset(eps_sb, LN_EPS)

        # ---------------- pools (compute) ----------------
        xtp = ctx.enter_context(tc.tile_pool(name="xtp", bufs=1))
        qkv = ctx.enter_context(tc.tile_pool(name="qkv", bufs=2))
        kdp = ctx.enter_context(tc.tile_pool(name="kdp", bufs=1))
        qp = ctx.enter_context(tc.tile_pool(name="qp", bufs=1))
        op = ctx.enter_context(tc.tile_pool(name="op", bufs=1))
        ytp = ctx.enter_context(tc.tile_pool(name="ytp", bufs=2))
        small = ctx.enter_context(tc.tile_pool(name="small", bufs=2))
        mlp = ctx.enter_context(tc.tile_pool(name="mlp", bufs=1))
        wband = ctx.enter_context(tc.tile_pool(name="wband", bufs=4))
        w8band = ctx.enter_context(tc.tile_pool(name="w8band", bufs=2))
        stats = ctx.enter_context(tc.tile_pool(name="stats", bufs=1))

        # DRAM views: [128, K/128, COLS] (partition stride = 1 row)
        xT_r = {n: x.rearrange("(eo p) r -> p eo r", p=128)
                for n, x in xT_aps.items()}
        W_r = {k: v.rearrange("(ko p) c -> p ko c", p=128)
               for k, v in W.items()}
        out_r = out_ap.rearrange("(eo p) r -> p eo r", p=128)

        # ---------------- compute ----------------
        def proj(wT_r, srcs, outs, bias_sb=None, bias_col=0):
            """outs[si][:, c, :] = (wT[:, c*128:...]^T @ srcs[si]) (+ bias).
            Weight bands streamed 512 cols at a time (1KB descriptors)."""
            for cg in range(2):
                wb = wband.tile([128, EC, 512], BF16, tag="wb")
                nc.sync.dma_start(wb, wT_r[:, :, cg * 512:(cg + 1) * 512])
                for c4 in range(4):
                    c = cg * 4 + c4
                    psums = [mm_ps.tile([128, BSUB], F32, tag="mm",
                                        name=f"mm_{si}")
                             for si in range(len(srcs))]
                    for si, src in enumerate(srcs):
                        for k in range(EC):
                            _mm(nc, psums[si],
                                wb[:, k, c4 * 128:(c4 + 1) * 128],
                                src[:, k, :], (k == 0), (k == EC - 1))
                    for si in range(len(srcs)):
                        if bias_sb is not None:
                            nc.scalar.add(
                                outs[si][:, c, :], psums[si],
                                bias_sb[:, bias_col + c:bias_col + c + 1])
                        else:
                            nc.scalar.copy(outs[si][:, c, :], psums[si])

        def proj_fp8(wT_r, src8, out_t, bias_sb=None, bias_col=0):
            """fp8 DoubleRow projection: contracts two 128-chunks per matmul."""
            for cg in range(2):
                wb = w8band.tile([128, EC, 512], FP8, tag="w8")
                nc.sync.dma_start(wb, wT_r[:, :, cg * 512:(cg + 1) * 512])
                for c4 in range(4):
                    c = cg * 4 + c4
                    ps = mm_ps.tile([128, BSUB], F32, tag="mm")
                    for kk in range(EC // 2):
                        nc.tensor.matmul(
                            ps, lhsT=wb[:, 2 * kk:2 * kk + 2,
                                        c4 * 128:(c4 + 1) * 128],
                            rhs=src8[:, 2 * kk:2 * kk + 2, :],
                            start=(kk == 0), stop=(kk == EC // 2 - 1),
                            perf_mode=mybir.MatmulPerfMode.DoubleRow)
                    if bias_sb is not None:
                        nc.scalar.add(out_t[:, c, :], ps,
                                      bias_sb[:, bias_col + c:bias_col + c + 1])
                    else:
                        nc.scalar.copy(out_t[:, c, :], ps)

        def fuse_h(xvis_t, xtxt_t, YT):
            """YT = H_idtxt^T @ x_vis + H_vis^T @ x_txt + b_y.
            Independent of the gate path; fills the pass-start bubble."""
            for cg in range(2):
                hb = wband.tile([128, EC, 512], BF16, tag="wb")
                nc.sync.dma_start(
                    hb, W_r["h_idtxt"][:, :, cg * 512:(cg + 1) * 512])
                hb2 = wband.tile([128, EC, 512], BF16, tag="wb")
                nc.sync.dma_start(
                    hb2, W_r["h_vis"][:, :, cg * 512:(cg + 1) * 512])
                for c4 in range(4):
                    c = cg * 4 + c4
                    ps = mm_ps.tile([128, BSUB], F32, tag="mm")
                    for k in range(EC):
                        _mm(nc, ps, hb[:, k, c4 * 128:(c4 + 1) * 128],
                            xvis_t[:, k, :], (k == 0), False)
                    for k in range(EC):
                        _mm(nc, ps, hb2[:, k, c4 * 128:(c4 + 1) * 128],
                            xtxt_t[:, k, :], False, (k == EC - 1))
                    nc.scalar.add(YT[:, c, :], ps, by_sb[:, c:c + 1])

        def fuse_accum(name, gv_t, YT):
            """YT += G_name^T @ gv."""
            for cg in range(2):
                gb = wband.tile([128, EC, 512], BF16, tag="wb")
                nc.sync.dma_start(
                    gb, W_r[f"g_{name}"][:, :, cg * 512:(cg + 1) * 512])
                for c4 in range(4):
                    c = cg * 4 + c4
                    ps = mm_ps.tile([128, BSUB], F32, tag="mm")
                    for k in range(EC):
                        _mm(nc, ps, gb[:, k, c4 * 128:(c4 + 1) * 128],
                            gv_t[:, k, :], (k == 0), (k == EC - 1))
                    nc.vector.tensor_add(YT[:, c, :], ps, YT[:, c, :])

        def ln_out(YT, row0):
            """LayerNorm in feature-major space, store outT slice.

            Row stats via fp16 ones-matmul partition reductions; mu/rstd
            broadcast back across partitions with an outer-product matmul."""
            s12 = st_ps.tile([33, BSUB], F32, tag="s12")
            s1, s2 = s12[:1, :], s12[32:33, :]
            for c in range(EC):
                yc = small.tile([128, BSUB], FP16, tag="yc")
                nc.vector.tensor_copy(out=yc, in_=YT[:, c, :])
                _mm(nc, s1, ones_col, yc, (c == 0), (c == EC - 1))
            for c in range(EC):
                ysq = small.tile([128, BSUB], FP16, tag="ysq")
                nc.scalar.activation(ysq, YT[:, c, :], AF.Square)
                _mm(nc, s2, ones_col, ysq, (c == 0), (c == EC - 1))
            muh = stats.tile([1, BSUB], FP16, tag="muh")
            rsh = stats.tile([1, BSUB], FP16, tag="rsh")
            mu = stats.tile([1, BSUB], F32, tag="mu")
            nc.vector.tensor_scalar_mul(mu, s1, 1.0 / E)
            ex2 = stats.tile([1, BSUB], F32, tag="ex2")
            nc.vector.tensor_scalar_mul(ex2, s2, 1.0 / E)
            var = stats.tile([1, BSUB], F32, tag="var")
            nc.vector.tensor_mul(out=var, in0=mu, in1=mu)
            nc.vector.tensor_sub(var, ex2, var)
            std = stats.tile([1, BSUB], F32, tag="std")
            nc.scalar.activation(std, var, AF.Sqrt, bias=eps_sb[:1, :])
            with nc.allow_low_precision(reason="fp16 mu/rstd rows for the "
                                        "partition-broadcast matmul"):
                nc.vector.reciprocal(rsh, std)
                nc.vector.tensor_copy(out=muh, in_=mu)
            bc = at_ps.tile([128, BSUB], F32, tag="attn", name="bc_mu")
            _mm(nc, bc, ones_row, muh, True, True)
            bc2 = at_ps.tile([128, BSUB], F32, tag="attn", name="bc_rstd")
            _mm(nc, bc2, ones_row, rsh, True, True)
            for c in range(EC):
                nc.vector.tensor_sub(YT[:, c, :], YT[:, c, :], bc)
                nc.vector.tensor_mul(out=YT[:, c, :], in0=YT[:, c, :],
                                     in1=bc2)
                nc.vector.tensor_scalar(YT[:, c, :], YT[:, c, :],
                                        g_sb[:, c:c + 1], lb_sb[:, c:c + 1],
                                        ALU.mult, ALU.add)
            nc.sync.dma_start(out_r[:, :, row0:row0 + BSUB], YT)

        def phase_b(_it=None):
          pending_ln = None  # (YT, row0) of the previous pass
          for p in range(passes):
            row0 = p * BSUB
            # -- load transposed inputs (bf16 fuse/Vd paths, fp8 score path)
            XL = {}
            for n in SH_FEATS:
                dt = FP8 if n in SH_FP8 else BF16
                xt = xtp.tile([128, EC, BSUB], dt, tag=n)
                nc.sync.dma_start(xt, xT_r[n][:, :, row0:row0 + BSUB])
                XL[n] = xt

            YT = ytp.tile([128, EC, BSUB], F32, tag="yt")
            pend = None   # (name, VdT, wa) awaiting combine+fuse
            for ei, (name, qi, ai, bi) in enumerate(EXPERTS):
                # -- Q and Kd projections (fp8 DoubleRow, x64-scaled weights)
                QT = qp.tile([128, EC, BSUB], BF16, tag="qt")
                proj_fp8(W_r[f"wq8_{name}"], XL[f"xq8_{name}"], QT,
                         bias_in_sb[name], 0)
                KdT = kdp.tile([128, EC, BSUB], BF16, tag="kd")
                proj_fp8(W_r[f"wk8_{name}"], XL[f"xd8_{name}"], KdT)

                # -- pipelined LN of the previous pass, then the gate-free
                # H part of the fuse for this pass
                if ei == 0:
                    if pending_ln is not None:
                        ln_out(*pending_ln)
                        pending_ln = None
                    fuse_h(XL["xT_vis"], XL["xT_txt"], YT)

                # -- pipelined combine of previous expert: gv = g (.) Vd
                if pend is not None:
                    pn, pVd, pwa = pend
                    gv_t = op.tile([128, EC, BSUB], BF16, tag="gv")
                    for k in range(EC):
                        pse = at_ps.tile([128, BSUB], F32, tag="attn")
                        _mm(nc, pse, selt[:, k * 128:(k + 1) * 128], pwa,
                            True, True)
                        nc.vector.tensor_mul(out=gv_t[:, k, :], in0=pse,
                                             in1=pVd[:, k, :])

                # -- Vd projection
                VdT = qkv.tile([128, EC, BSUB], BF16, tag="vd")
                proj(W_r[f"wv_{name}"], [XL[f"xdT_{name}"]], [VdT])

                # -- pipelined fuse of previous expert
                if pend is not None:
                    fuse_accum(pn, gv_t, YT)

                # -- score diff + gate: wa = sigmoid((q . kd)/8)
                wa = small.tile([16, BSUB], BF16, tag="wa")
                ps = at_ps.tile([128, BSUB], F32, tag="attn")
                multt = mlp.tile([128, EC, BSUB], BF16, tag="multt")
                for k in range(EC):
                    nc.vector.tensor_mul(out=multt[:, k, :], in0=QT[:, k, :],
                                         in1=KdT[:, k, :])
                for k in range(EC):
                    _mm(nc, ps[:16, :], sel[:, k * 16:(k + 1) * 16],
                        multt[:, k, :], (k == 0), (k == EC - 1))
                nc.scalar.activation(wa, ps[:16, :], AF.Sigmoid,
                                     scale=0.125 / (W8SCALE * W8SCALE))
                pend = (name, VdT, wa)

            # -- tail: combine + fuse of the last expert
            pn, pVd, pwa = pend
            gv_t = op.tile([128, EC, BSUB], BF16, tag="gv")
            for k in range(EC):
                pse = at_ps.tile([128, BSUB], F32, tag="attn")
                _mm(nc, pse, selt[:, k * 128:(k + 1) * 128], pwa, True, True)
                nc.vector.tensor_mul(out=gv_t[:, k, :], in0=pse,
                                     in1=pVd[:, k, :])
            fuse_accum(pn, gv_t, YT)
            pending_ln = (YT, row0)

          ln_out(*pending_ln)

        if iters == 1:
            phase_b()
        else:
            with tc.For_i(0, iters, 1) as _i:
                phase_b(_i)


# ---------------- host-side input prep ----------------

def prepare_full_inputs(inputs):
    """Full (unsharded) kernel inputs from the reference input dict.

    Sharded tensors (SH_FEATS) are stacked core-major on axis 0:
    [NCORES*E, BC]; everything else is replicated as-is."""
    import ml_dtypes
    BF = ml_dtypes.bfloat16
    F8 = mybir.dt.np(FP8)
    out = {}
    feats = {n: np.asarray(inputs[f"{n}_feat"], np.float32)
             for n, _, _, _ in EXPERTS}
    names = [n for n, _, _, _ in EXPERTS]

    def shard_T(x, dt=BF):
        xT = x.T.astype(dt)                                 # [E, B] contig
        sh = xT.reshape(E, NCORES, BC).swapaxes(0, 1)       # [NC, E, BC]
        return np.ascontiguousarray(sh).reshape(NCORES * E, BC)

    for name, _, ai, bi in EXPERTS:
        xd = feats[names[ai]] - feats[names[bi]]
        if f"xT_{name}" in SH_BF16:
            out[f"xT_{name}"] = shard_T(feats[name])
        out[f"xdT_{name}"] = shard_T(xd)
        out[f"xq8_{name}"] = shard_T(feats[name], F8)
        out[f"xd8_{name}"] = shard_T(xd, F8)

    wf = np.asarray(inputs["w_fuse"], np.float32)           # [E, 3E]
    b_y = np.asarray(inputs["b_fuse"], np.float32).copy()
    h_parts = {}
    for i, (name, _, _, _) in enumerate(EXPERTS):
        w_in = np.asarray(inputs[f"w_in_{name}"], np.float32)   # [3E, E]
        w_out = np.asarray(inputs[f"w_out_{name}"], np.float32)  # [E, E]
        b_in = np.asarray(inputs[f"b_in_{name}"], np.float32)
        out[f"wq8T_{name}"] = np.ascontiguousarray(
            w_in[:E].T * W8SCALE).astype(F8)
        out[f"wk8T_{name}"] = np.ascontiguousarray(
            w_in[E:2 * E].T * W8SCALE).astype(F8)
        out[f"wvT_{name}"] = np.ascontiguousarray(w_in[2 * E:].T).astype(BF)
        wf_i = wf[:, i * E:(i + 1) * E]                      # [E, E]
        g_i = wf_i @ w_out                                   # [E, E] fp32
        h_i = g_i @ w_in[2 * E:]                             # G_i @ Wv_i
        out[f"gT_{name}"] = np.ascontiguousarray(g_i.T).astype(BF)
        h_parts[name] = h_i
        b_in_dev = b_in.copy()
        b_in_dev[:E] *= W8SCALE      # bq joins the x64-scaled fp8 Q
        out[f"b_in_{name}"] = b_in_dev
        b_y += wf_i @ np.asarray(inputs[f"b_out_{name}"], np.float32)
        b_y += g_i @ b_in[2 * E:]
    out["b_y"] = b_y
    out["hT_idtxt"] = np.ascontiguousarray(
        (h_parts["id"] + h_parts["txt"]).T).astype(BF)
    out["hT_vis"] = np.ascontiguousarray(h_parts["vis"].T).astype(BF)
    out["ln_g"] = np.asarray(inputs["ln_g"], np.float32)
    out["ln_b"] = np.asarray(inputs["ln_b"], np.float32)
    return out


def core_shard(full, core):
    """Per-core input dict from prepare_full_inputs() output."""
    m = {}
    for k, v in full.items():
        if k in SH_FEATS:
            m[k] = v[core * E:(core + 1) * E]
        else:
            m[k] = v
    return m


_NC_CACHE = {}


def _get_program():
    if "nc" not in _NC_CACHE:
        _NC_CACHE["nc"] = build_program()
    return _NC_CACHE["nc"]


def _get_runner():
    """Cached jitted SPMD runner. Feats/outputs sharded over cores, weights
    replicated (sent once, not 8x)."""
    if "runner" in _NC_CACHE:
        return _NC_CACHE["runner"]
    import jax
    from jax.sharding import Mesh, PartitionSpec as P
    from jax.experimental.shard_map import shard_map
    from concourse.bass2jax import (_bass_exec_p, install_neuronx_cc_hook,
                                    partition_id_tensor)

    nc = _get_program()
    install_neuronx_cc_hook()
    assert nc.dbg_addr is None
    pid_name = (nc.partition_id_tensor.name
                if nc.partition_id_tensor is not None else None)

    in_names, out_names, out_avals = [], [], []
    for alloc in nc.m.functions[0].allocations:
        if not isinstance(alloc, mybir.MemoryLocationSet):
            continue
        name = alloc.memorylocations[0].name
        if alloc.kind == "ExternalInput":
            if name != pid_name:
                in_names.append(name)
        elif alloc.kind == "ExternalOutput":
            out_names.append(name)
            out_avals.append(jax.core.ShapedArray(
                tuple(alloc.tensor_shape), mybir.dt.np(alloc.dtype)))
    n_params = len(in_names)

    all_in_names = in_names + out_names + ([pid_name] if pid_name else [])

    def _body(*args):
        operands = list(args)
        if pid_name is not None:
            operands.append(partition_id_tensor())
        outs = _bass_exec_p.bind(
            *operands,
            out_avals=tuple(out_avals),
            in_names=tuple(all_in_names),
            out_names=tuple(out_names),
            lowering_input_output_aliases=(),
            sim_require_finite=True,
            sim_require_nnan=True,
            nc=nc,
        )
        return tuple(outs)

    devices = jax.devices()[:NCORES]
    mesh = Mesh(np.asarray(devices), ("core",))
    in_specs = tuple(P("core") if n in SH_FEATS else P() for n in in_names) + \
        (P("core"),) * len(out_names)
    out_specs = (P("core"),) * len(out_names)
    sharded = jax.jit(
        shard_map(_body, mesh=mesh, in_specs=in_specs, out_specs=out_specs,
                  check_rep=False),
        donate_argnums=tuple(range(n_params, n_params + len(out_names))),
        keep_unused=True)
    _NC_CACHE["runner"] = (sharded, in_names, out_names, out_avals)
    return _NC_CACHE["runner"]


def postprocess_output(outT_stacked):
    """[NCORES*E, BC] feature-major per-core slabs -> [B, E] f32."""
    o = np.asarray(outT_stacked).astype(np.float32)
    o = o.reshape(NCORES, E, BC).transpose(0, 2, 1)
    return np.ascontiguousarray(o).reshape(B, E)


def kernel(**inputs):
    full = prepare_full_inputs(inputs)
    sharded, in_names, out_names, out_avals = _get_runner()
    args = [full[n] for n in in_names]
    zeros = [np.zeros((NCORES * a.shape[0], *a.shape[1:]), a.dtype)
             for a in out_avals]
    outs = sharded(*args, *zeros)
    return postprocess_output(outs[0])


# revision 73
# speedup vs baseline: 1.0056x; 1.0056x over previous
"""Trainium2 Bass kernel for nn_CrossAttentionExpert.

Three single-query cross-attention "experts" (id/txt/vis), each attending over
the other two modalities (K=2 kv positions), outputs concatenated, fused by a
linear layer and LayerNorm.

Strategy: pure data parallel over 8 NeuronCores, batch 16384 -> 2048 rows/core.
Weights replicated.

Math per expert (K=2 softmax == sigmoid gate on the score difference):
  g   = sigmoid(q . (k_a - k_b) / sqrt(D))    per head
  o   = v_b + g * (v_a - v_b)
Since k_a - k_b = Wk (x_a - x_b) and v_a - v_b = Wv (x_a - x_b) (biases
cancel), and with the output projection + fuse folded host-side
(G_i = Wf_i @ Wout_i, H_i = G_i @ Wv_i, b_y collecting all biases):
  y = sum_i [ H_i x_{b_i} + G_i (g_i (.) Wv_i xd_i) ] + b_y
the device does, per expert, only Q (fp8), Kd (fp8), Vd (bf16) projections
plus the shared H/G fuse GEMMs.  Experts id+txt share x_b = x_vis, so their
H matrices are pre-summed (H_idtxt).  xd = x_a - x_b is precomputed host-side.

The score path (Q, Kd) runs in fp8 e4m3 with DoubleRow matmuls (two 128-deep
k-chunks per instruction); weights are pre-scaled by 64 to avoid e4m3
subnormals and the 1/64^2 is folded into the sigmoid scale.  Score noise only
perturbs the sigmoid gate, keeping the end-to-end error ~1.7e-2.

Host-side prep (numpy, outside NEFF): weight slices transposed + cast to
bf16/fp8 in the exact DRAM layout the kernel streams; input features (and
pair differences) transposed to feature-major [E, BC] per core.  The output
is produced feature-major [E, BC] per core (LayerNorm done in transposed
space: row stats via fp16 ones-matmul partition reductions, mu/rstd
broadcast back via outer-product matmuls) and transposed on the host.

On-core dataflow: feature-major activations; weights are the stationary
operand (fp32 PSUM accumulation).  2048 rows/core in 4 passes of 512, with
the previous expert's gate/fuse and the previous pass's LayerNorm software-
pipelined under the current expert's projections.  Per-head score reduction
and gate broadcast use small constant selector matmuls on the PE.
"""

import sys

sys.path.insert(0, "/opt/trn_rl_repo")

import numpy as np

import concourse.bass as bass
import concourse.bacc as bacc
import concourse.mybir as mybir
import concourse.tile as tile
from concourse.masks import make_identity

F32 = mybir.dt.float32
BF16 = mybir.dt.bfloat16
FP16 = mybir.dt.float16
FP8 = mybir.dt.float8e4
W8SCALE = 64.0          # host-side scale on fp8 Wq/Wk (avoid e4m3 subnormals)
AF = mybir.ActivationFunctionType
ALU = mybir.AluOpType
AX = mybir.AxisListType

B, E, H, D = 16384, 1024, 16, 64
NCORES = 8
BC = B // NCORES          # 2048 rows per core
BSUB = 512                # rows per pass
NPASS = BC // BSUB        # 4
EC = E // 128             # 8 feature chunks
LN_EPS = 1e-5

EXPERTS = [  # (name, q_idx, kv_a_idx, kv_b_idx)
    ("id", 0, 1, 2),
    ("txt", 1, 0, 2),
    ("vis", 2, 0, 1),
]
# sharded inputs (stacked core-major on axis 0).  bf16: feats txt/vis (fuse
# H-path) + 3 pair diffs (Vd); fp8: per-expert q-feat + pair diff (score path).
SH_BF16 = ["xT_txt", "xT_vis", "xdT_id", "xdT_txt", "xdT_vis"]
SH_FP8 = ["xq8_id", "xq8_txt", "xq8_vis", "xd8_id", "xd8_txt", "xd8_vis"]
SH_FEATS = SH_BF16 + SH_FP8


def _build_selectors(nc, sel, selt):
    """sel: [128, 8*16] bf16, chunk k cols k*16:(k+1)*16:
         sel_k[d, h] = 1 iff h == 2k + d//64   (score head-reduce, lhsT)
       selt: [16, 8*128] bf16, chunk k cols k*128:(k+1)*128:
         selt_k[h, d] = 1 iff h == 2k + d//64  (gate head-broadcast, lhsT)
    Condition h == 2k + d//64  <=>  -63 <= 64h - 128k - d <= 0.
    """
    nc.gpsimd.memset(sel, 1.0)
    nc.gpsimd.memset(selt, 1.0)
    for k in range(8):
        s = sel[:, k * 16:(k + 1) * 16]
        nc.gpsimd.affine_select(out=s, in_=s, compare_op=ALU.is_ge, fill=0.0,
                                base=128 * k, pattern=[[-64, 16]],
                                channel_multiplier=1)
        nc.gpsimd.affine_select(out=s, in_=s, compare_op=ALU.is_ge, fill=0.0,
                                base=63 - 128 * k, pattern=[[64, 16]],
                                channel_multiplier=-1)
        t = selt[:, k * 128:(k + 1) * 128]
        nc.gpsimd.affine_select(out=t, in_=t, compare_op=ALU.is_ge, fill=0.0,
                                base=128 * k, pattern=[[1, 128]],
                                channel_multiplier=-64)
        nc.gpsimd.affine_select(out=t, in_=t, compare_op=ALU.is_ge, fill=0.0,
                                base=63 - 128 * k, pattern=[[-1, 128]],
                                channel_multiplier=64)


def _mm(nc, out, lhsT, rhs, start, stop):
    nc.tensor.matmul(out, lhsT=lhsT, rhs=rhs, start=start, stop=stop)


def build_program(iters=1, passes=NPASS):
    nc = bacc.Bacc("TRN2", target_bir_lowering=False, debug=False,
                   num_devices=NCORES)

    xT_aps = {n: nc.dram_tensor(n, [E, BC], BF16, kind="ExternalInput").ap()
              for n in SH_BF16}
    xT_aps.update({n: nc.dram_tensor(n, [E, BC], FP8,
                                     kind="ExternalInput").ap()
                   for n in SH_FP8})
    W = {}
    for name, _, _, _ in EXPERTS:
        for kind in ("wv", "g"):
            W[f"{kind}_{name}"] = nc.dram_tensor(
                f"{kind}T_{name}", [E, E], BF16, kind="ExternalInput").ap()
        for kind in ("wq8", "wk8"):
            W[f"{kind}_{name}"] = nc.dram_tensor(
                f"{kind}T_{name}", [E, E], FP8, kind="ExternalInput").ap()
    # combined H matrices: experts id+txt share x_b = vis; vis uses x_b = txt
    W["h_idtxt"] = nc.dram_tensor("hT_idtxt", [E, E], BF16,
                                  kind="ExternalInput").ap()
    W["h_vis"] = nc.dram_tensor("hT_vis", [E, E], BF16,
                                kind="ExternalInput").ap()
    b_in = {n: nc.dram_tensor(f"b_in_{n}", [3 * E], F32,
                              kind="ExternalInput").ap()
            for n, _, _, _ in EXPERTS}
    b_y = nc.dram_tensor("b_y", [E], F32, kind="ExternalInput").ap()
    ln_g = nc.dram_tensor("ln_g", [E], F32, kind="ExternalInput").ap()
    ln_b = nc.dram_tensor("ln_b", [E], F32, kind="ExternalInput").ap()
    out_ap = nc.dram_tensor("outT", [E, BC], F32,
                            kind="ExternalOutput").ap()

    with tile.TileContext(nc) as tc:
        _emit(nc, tc, xT_aps, W, b_in, b_y, ln_g, ln_b, out_ap, iters, passes)
    nc.compile()
    return nc


def _emit(nc, tc, xT_aps, W, b_in, b_y, ln_g, ln_b, out_ap,
          iters=1, passes=NPASS):
    from contextlib import ExitStack
    ctx = ExitStack()
    with ctx:
        # ---------------- pools ----------------
        consts = ctx.enter_context(tc.tile_pool(name="consts", bufs=1))
        mm_ps = ctx.enter_context(tc.tile_pool(name="mm_ps", bufs=4, space="PSUM"))
        at_ps = ctx.enter_context(tc.tile_pool(name="at_ps", bufs=3, space="PSUM"))
        st_ps = ctx.enter_context(tc.tile_pool(name="st_ps", bufs=1, space="PSUM"))

        # ---------------- constants ----------------
        ones_col = consts.tile([128, 1], FP16, tag="ones_col")
        nc.vector.memset(ones_col, 1.0)
        ones_row = consts.tile([1, 128], FP16, tag="ones_row")
        nc.vector.memset(ones_row, 1.0)
        sel = consts.tile([128, 8 * 16], BF16, tag="sel")
        selt = consts.tile([16, 8 * 128], BF16, tag="selt")
        _build_selectors(nc, sel, selt)

        bias_in_sb = {}
        for name, _, _, _ in EXPERTS:
            t = consts.tile([128, 24], F32, tag=f"bin_{name}")
            nc.gpsimd.dma_start(t, b_in[name].rearrange("(c p) -> p c", p=128))
            bias_in_sb[name] = t
        by_sb = consts.tile([128, 8], F32, tag="b_y")
        nc.gpsimd.dma_start(by_sb, b_y.rearrange("(c p) -> p c", p=128))

        g_sb = consts.tile([128, 8], F32, tag="ln_g")
        nc.gpsimd.dma_start(g_sb, ln_g.rearrange("(c p) -> p c", p=128))
        lb_sb = consts.tile([128, 8], F32, tag="ln_b")
        nc.gpsimd.dma_start(lb_sb, ln_b.rearrange("(c p) -> p c", p=128))
        eps_sb = consts.tile([128, 1], F32, tag="eps")
        nc.vector.memset(eps_sb, LN_EPS)

        # ---------------- pools (compute) ----------------
        xtp = ctx.enter_context(tc.tile_pool(name="xtp", bufs=1))
        qkv = ctx.enter_context(tc.tile_pool(name="qkv", bufs=2))
        kdp = ctx.enter_context(tc.tile_pool(name="kdp", bufs=1))
        qp = ctx.enter_context(tc.tile_pool(name="qp", bufs=1))
        op = ctx.enter_context(tc.tile_pool(name="op", bufs=1))
        ytp = ctx.enter_context(tc.tile_pool(name="ytp", bufs=2))
        small = ctx.enter_context(tc.tile_pool(name="small", bufs=2))
        mlp = ctx.enter_context(tc.tile_pool(name="mlp", bufs=1))
        wband = ctx.enter_context(tc.tile_pool(name="wband", bufs=4))
        w8band = ctx.enter_context(tc.tile_pool(name="w8band", bufs=2))
        stats = ctx.enter_context(tc.tile_pool(name="stats", bufs=1))

        # DRAM views: [128, K/128, COLS] (partition stride = 1 row)
        xT_r = {n: x.rearrange("(eo p) r -> p eo r", p=128)
                for n, x in xT_aps.items()}
        W_r = {k: v.rearrange("(ko p) c -> p ko c", p=128)
               for k, v in W.items()}
        out_r = out_ap.rearrange("(eo p) r -> p eo r", p=128)

        # ---------------- compute ----------------
        def proj(wT_r, srcs, outs, bias_sb=None, bias_col=0):
            """outs[si][:, c, :] = (wT[:, c*128:...]^T @ srcs[si]) (+ bias).
            Weight bands streamed 512 cols at a time (1KB descriptors)."""
            for cg in range(2):
                wb = wband.tile([128, EC, 512], BF16, tag="wb")
                nc.sync.dma_start(wb, wT_r[:, :, cg * 512:(cg + 1) * 512])
                for c4 in range(4):
                    c = cg * 4 + c4
                    psums = [mm_ps.tile([128, BSUB], F32, tag="mm",
                                        name=f"mm_{si}")
                             for si in range(len(srcs))]
                    for si, src in enumerate(srcs):
                        for k in range(EC):
                            _mm(nc, psums[si],
                                wb[:, k, c4 * 128:(c4 + 1) * 128],
                                src[:, k, :], (k == 0), (k == EC - 1))
                    for si in range(len(srcs)):
                        if bias_sb is not None:
                            nc.scalar.add(
                                outs[si][:, c, :], psums[si],
                                bias_sb[:, bias_col + c:bias_col + c + 1])
                        else:
                            nc.scalar.copy(outs[si][:, c, :], psums[si])

        def proj_fp8(wT_r, src8, out_t, bias_sb=None, bias_col=0):
            """fp8 DoubleRow projection: contracts two 128-chunks per matmul."""
            for cg in range(2):
                wb = w8band.tile([128, EC, 512], FP8, tag="w8")
                nc.sync.dma_start(wb, wT_r[:, :, cg * 512:(cg + 1) * 512])
                for c4 in range(4):
                    c = cg * 4 + c4
                    ps = mm_ps.tile([128, BSUB], F32, tag="mm")
                    for kk in range(EC // 2):
                        nc.tensor.matmul(
                            ps, lhsT=wb[:, 2 * kk:2 * kk + 2,
                                        c4 * 128:(c4 + 1) * 128],
                            rhs=src8[:, 2 * kk:2 * kk + 2, :],
                            start=(kk == 0), stop=(kk == EC // 2 - 1),
                            perf_mode=mybir.MatmulPerfMode.DoubleRow)
                    if bias_sb is not None:
                        nc.scalar.add(out_t[:, c, :], ps,
                                      bias_sb[:, bias_col + c:bias_col + c + 1])
                    else:
                        nc.scalar.copy(out_t[:, c, :], ps)

        def fuse_h(xvis_t, xtxt_t, YT):
            """YT = H_idtxt^T @ x_vis + H_vis^T @ x_txt + b_y.
            Independent of the gate path; fills the pass-start bubble."""
            for cg in range(2):
                hb = wband.tile([128, EC, 512], BF16, tag="wb")
                nc.sync.dma_start(
                    hb, W_r["h_idtxt"][:, :, cg * 512:(cg + 1) * 512])
                hb2 = wband.tile([128, EC, 512], BF16, tag="wb")
                nc.sync.dma_start(
                    hb2, W_r["h_vis"][:, :, cg * 512:(cg + 1) * 512])
                for c4 in range(4):
                    c = cg * 4 + c4
                    ps = mm_ps.tile([128, BSUB], F32, tag="mm")
                    for k in range(EC):
                        _mm(nc, ps, hb[:, k, c4 * 128:(c4 + 1) * 128],
                            xvis_t[:, k, :], (k == 0), False)
                    for k in range(EC):
                        _mm(nc, ps, hb2[:, k, c4 * 128:(c4 + 1) * 128],
                            xtxt_t[:, k, :], False, (k == EC - 1))
                    nc.scalar.add(YT[:, c, :], ps, by_sb[:, c:c + 1])

        def fuse_accum(name, gv_t, YT):
            """YT += G_name^T @ gv."""
            for cg in range(2):
                gb = wband.tile([128, EC, 512], BF16, tag="wb")
                nc.sync.dma_start(
                    gb, W_r[f"g_{name}"][:, :, cg * 512:(cg + 1) * 512])
                for c4 in range(4):
                    c = cg * 4 + c4
                    ps = mm_ps.tile([128, BSUB], F32, tag="mm")
                    for k in range(EC):
                        _mm(nc, ps, gb[:, k, c4 * 128:(c4 + 1) * 128],
                            gv_t[:, k, :], (k == 0), (k == EC - 1))
                    nc.vector.tensor_add(YT[:, c, :], ps, YT[:, c, :])

        def ln_out(YT, row0):
            """LayerNorm in feature-major space, store outT slice.

            Row stats via fp16 ones-matmul partition reductions; mu/rstd
            broadcast back across partitions with an outer-product matmul."""
            s12 = st_ps.tile([33, BSUB], F32, tag="s12")
            s1, s2 = s12[:1, :], s12[32:33, :]
            for c in range(EC):
                yc = small.tile([128, BSUB], FP16, tag="yc")
                nc.vector.tensor_copy(out=yc, in_=YT[:, c, :])
                _mm(nc, s1, ones_col, yc, (c == 0), (c == EC - 1))
            for c in range(EC):
                ysq = small.tile([128, BSUB], FP16, tag="ysq")
                nc.scalar.activation(ysq, YT[:, c, :], AF.Square)
                _mm(nc, s2, ones_col, ysq, (c == 0), (c == EC - 1))
            muh = stats.tile([1, BSUB], FP16, tag="muh")
            rsh = stats.tile([1, BSUB], FP16, tag="rsh")
            mu = stats.tile([1, BSUB], F32, tag="mu")
            nc.vector.tensor_scalar_mul(mu, s1, 1.0 / E)
            ex2 = stats.tile([1, BSUB], F32, tag="ex2")
            nc.vector.tensor_scalar_mul(ex2, s2, 1.0 / E)
            var = stats.tile([1, BSUB], F32, tag="var")
            nc.vector.tensor_mul(out=var, in0=mu, in1=mu)
            nc.vector.tensor_sub(var, ex2, var)
            std = stats.tile([1, BSUB], F32, tag="std")
            nc.scalar.activation(std, var, AF.Sqrt, bias=eps_sb[:1, :])
            with nc.allow_low_precision(reason="fp16 mu/rstd rows for the "
                                        "partition-broadcast matmul"):
                nc.vector.reciprocal(rsh, std)
                nc.vector.tensor_copy(out=muh, in_=mu)
            bc = at_ps.tile([128, BSUB], F32, tag="attn", name="bc_mu")
            _mm(nc, bc, ones_row, muh, True, True)
            bc2 = at_ps.tile([128, BSUB], F32, tag="attn", name="bc_rstd")
            _mm(nc, bc2, ones_row, rsh, True, True)
            for c in range(EC):
                nc.vector.tensor_sub(YT[:, c, :], YT[:, c, :], bc)
                nc.vector.tensor_mul(out=YT[:, c, :], in0=YT[:, c, :],
                                     in1=bc2)
                nc.vector.tensor_scalar(YT[:, c, :], YT[:, c, :],
                                        g_sb[:, c:c + 1], lb_sb[:, c:c + 1],
                                        ALU.mult, ALU.add)
            nc.sync.dma_start(out_r[:, :, row0:row0 + BSUB], YT)

        def phase_b(_it=None):
          pending_ln = None  # (YT, row0) of the previous pass
          for p in range(passes):
            row0 = p * BSUB
            # -- load transposed inputs (bf16 fuse/Vd paths, fp8 score path)
            XL = {}
            for n in SH_FEATS:
                dt = FP8 if n in SH_FP8 else BF16
                xt = xtp.tile([128, EC, BSUB], dt, tag=n)
                nc.sync.dma_start(xt, xT_r[n][:, :, row0:row0 + BSUB])
                XL[n] = xt

            YT = ytp.tile([128, EC, BSUB], F32, tag="yt")
            pend = None   # (name, VdT, wa) awaiting combine+fuse
            for ei, (name, qi, ai, bi) in enumerate(EXPERTS):
                # -- Q and Kd projections (fp8 DoubleRow, x64-scaled weights)
                QT = qp.tile([128, EC, BSUB], BF16, tag="qt")
                proj_fp8(W_r[f"wq8_{name}"], XL[f"xq8_{name}"], QT,
                         bias_in_sb[name], 0)
                KdT = kdp.tile([128, EC, BSUB], BF16, tag="kd")
                proj_fp8(W_r[f"wk8_{name}"], XL[f"xd8_{name}"], KdT)

                # -- pipelined LN of the previous pass, then the gate-free
                # H part of the fuse for this pass
                if ei == 0:
                    if pending_ln is not None:
                        ln_out(*pending_ln)
                        pending_ln = None
                    fuse_h(XL["xT_vis"], XL["xT_txt"], YT)

                # -- pipelined combine of previous expert: gv = g (.) Vd
                if pend is not None:
                    pn, pVd, pwa = pend
                    gv_t = op.tile([128, EC, BSUB], BF16, tag="gv")
                    for k in range(EC):
                        pse = at_ps.tile([128, BSUB], F32, tag="attn")
                        _mm(nc, pse, selt[:, k * 128:(k + 1) * 128], pwa,
                            True, True)
                        nc.vector.tensor_mul(out=gv_t[:, k, :], in0=pse,
                                             in1=pVd[:, k, :])

                # -- Vd projection
                VdT = qkv.tile([128, EC, BSUB], BF16, tag="vd")
                proj(W_r[f"wv_{name}"], [XL[f"xdT_{name}"]], [VdT])

                # -- pipelined fuse of previous expert
                if pend is not None:
                    fuse_accum(pn, gv_t, YT)

                # -- score diff + gate: wa = sigmoid((q . kd)/8)
                wa = small.tile([16, BSUB], BF16, tag="wa")
                ps = at_ps.tile([128, BSUB], F32, tag="attn")
                multt = mlp.tile([128, EC, BSUB], BF16, tag="multt")
                for k in range(EC):
                    nc.vector.tensor_mul(out=multt[:, k, :], in0=QT[:, k, :],
                                         in1=KdT[:, k, :])
                for k in range(EC):
                    _mm(nc, ps[:16, :], sel[:, k * 16:(k + 1) * 16],
                        multt[:, k, :], (k == 0), (k == EC - 1))
                nc.scalar.activation(wa, ps[:16, :], AF.Sigmoid,
                                     scale=0.125 / (W8SCALE * W8SCALE))
                pend = (name, VdT, wa)

            # -- tail: combine + fuse of the last expert
            pn, pVd, pwa = pend
            gv_t = op.tile([128, EC, BSUB], BF16, tag="gv")
            for k in range(EC):
                pse = at_ps.tile([128, BSUB], F32, tag="attn")
                _mm(nc, pse, selt[:, k * 128:(k + 1) * 128], pwa, True, True)
                nc.vector.tensor_mul(out=gv_t[:, k, :], in0=pse,
                                     in1=pVd[:, k, :])
            fuse_accum(pn, gv_t, YT)
            pending_ln = (YT, row0)

          ln_out(*pending_ln)

        if iters == 1:
            phase_b()
        else:
            with tc.For_i(0, iters, 1) as _i:
                phase_b(_i)


# ---------------- host-side input prep ----------------

def prepare_full_inputs(inputs):
    """Full (unsharded) kernel inputs from the reference input dict.

    Sharded tensors (SH_FEATS) are stacked core-major on axis 0:
    [NCORES*E, BC]; everything else is replicated as-is."""
    import ml_dtypes
    BF = ml_dtypes.bfloat16
    F8 = mybir.dt.np(FP8)
    out = {}
    feats = {n: np.asarray(inputs[f"{n}_feat"], np.float32)
             for n, _, _, _ in EXPERTS}
    names = [n for n, _, _, _ in EXPERTS]

    def shard_T(x, dt=BF):
        xT = x.T.astype(dt)                                 # [E, B] contig
        sh = xT.reshape(E, NCORES, BC).swapaxes(0, 1)       # [NC, E, BC]
        return np.ascontiguousarray(sh).reshape(NCORES * E, BC)

    for name, _, ai, bi in EXPERTS:
        xd = feats[names[ai]] - feats[names[bi]]
        if f"xT_{name}" in SH_BF16:
            out[f"xT_{name}"] = shard_T(feats[name])
        out[f"xdT_{name}"] = shard_T(xd)
        out[f"xq8_{name}"] = shard_T(feats[name], F8)
        out[f"xd8_{name}"] = shard_T(xd, F8)

    wf = np.asarray(inputs["w_fuse"], np.float32)           # [E, 3E]
    b_y = np.asarray(inputs["b_fuse"], np.float32).copy()
    h_parts = {}
    for i, (name, _, _, _) in enumerate(EXPERTS):
        w_in = np.asarray(inputs[f"w_in_{name}"], np.float32)   # [3E, E]
        w_out = np.asarray(inputs[f"w_out_{name}"], np.float32)  # [E, E]
        b_in = np.asarray(inputs[f"b_in_{name}"], np.float32)
        out[f"wq8T_{name}"] = np.ascontiguousarray(
            w_in[:E].T * W8SCALE).astype(F8)
        out[f"wk8T_{name}"] = np.ascontiguousarray(
            w_in[E:2 * E].T * W8SCALE).astype(F8)
        out[f"wvT_{name}"] = np.ascontiguousarray(w_in[2 * E:].T).astype(BF)
        wf_i = wf[:, i * E:(i + 1) * E]                      # [E, E]
        g_i = wf_i @ w_out                                   # [E, E] fp32
        h_i = g_i @ w_in[2 * E:]                             # G_i @ Wv_i
        out[f"gT_{name}"] = np.ascontiguousarray(g_i.T).astype(BF)
        h_parts[name] = h_i
        b_in_dev = b_in.copy()
        b_in_dev[:E] *= W8SCALE      # bq joins the x64-scaled fp8 Q
        out[f"b_in_{name}"] = b_in_dev
        b_y += wf_i @ np.asarray(inputs[f"b_out_{name}"], np.float32)
        b_y += g_i @ b_in[2 * E:]
    out["b_y"] = b_y
    out["hT_idtxt"] = np.ascontiguousarray(
        (h_parts["id"] + h_parts["txt"]).T).astype(BF)
    out["hT_vis"] = np.ascontiguousarray(h_parts["vis"].T).astype(BF)
    out["ln_g"] = np.asarray(inputs["ln_g"], np.float32)
    out["ln_b"] = np.asarray(inputs["ln_b"], np.float32)
    return out


def core_shard(full, core):
    """Per-core input dict from prepare_full_inputs() output."""
    m = {}
    for k, v in full.items():
        if k in SH_FEATS:
            m[k] = v[core * E:(core + 1) * E]
        else:
            m[k] = v
    return m


_NC_CACHE = {}


def _get_program():
    if "nc" not in _NC_CACHE:
        _NC_CACHE["nc"] = build_program()
    return _NC_CACHE["nc"]


def _get_runner():
    """Cached jitted SPMD runner. Feats/outputs sharded over cores, weights
    replicated (sent once, not 8x)."""
    if "runner" in _NC_CACHE:
        return _NC_CACHE["runner"]
    import jax
    from jax.sharding import Mesh, PartitionSpec as P
    from jax.experimental.shard_map import shard_map
    from concourse.bass2jax import (_bass_exec_p, install_neuronx_cc_hook,
                                    partition_id_tensor)

    nc = _get_program()
    install_neuronx_cc_hook()
    assert nc.dbg_addr is None
    pid_name = (nc.partition_id_tensor.name
                if nc.partition_id_tensor is not None else None)

    in_names, out_names, out_avals = [], [], []
    for alloc in nc.m.functions[0].allocations:
        if not isinstance(alloc, mybir.MemoryLocationSet):
            continue
        name = alloc.memorylocations[0].name
        if alloc.kind == "ExternalInput":
            if name != pid_name:
                in_names.append(name)
        elif alloc.kind == "ExternalOutput":
            out_names.append(name)
            out_avals.append(jax.core.ShapedArray(
                tuple(alloc.tensor_shape), mybir.dt.np(alloc.dtype)))
    n_params = len(in_names)

    all_in_names = in_names + out_names + ([pid_name] if pid_name else [])

    def _body(*args):
        operands = list(args)
        if pid_name is not None:
            operands.append(partition_id_tensor())
        outs = _bass_exec_p.bind(
            *operands,
            out_avals=tuple(out_avals),
            in_names=tuple(all_in_names),
            out_names=tuple(out_names),
            lowering_input_output_aliases=(),
            sim_require_finite=True,
            sim_require_nnan=True,
            nc=nc,
        )
        return tuple(outs)

    devices = jax.devices()[:NCORES]
    mesh = Mesh(np.asarray(devices), ("core",))
    in_specs = tuple(P("core") if n in SH_FEATS else P() for n in in_names) + \
        (P("core"),) * len(out_names)
    out_specs = (P("core"),) * len(out_names)
    sharded = jax.jit(
        shard_map(_body, mesh=mesh, in_specs=in_specs, out_specs=out_specs,
                  check_rep=False),
        donate_argnums=tuple(range(n_params, n_params + len(out_names))),
        keep_unused=True)
    _NC_CACHE["runner"] = (sharded, in_names, out_names, out_avals)
    return _NC_CACHE["runner"]


def postprocess_output(outT_stacked):
    """[NCORES*E, BC] feature-major per-core slabs -> [B, E] f32."""
    o = np.asarray(outT_stacked).astype(np.float32)
    o = o.reshape(NCORES, E, BC).transpose(0, 2, 1)
    return np.ascontiguousarray(o).reshape(B, E)


def kernel(**inputs):
    full = prepare_full_inputs(inputs)
    sharded, in_names, out_names, out_avals = _get_runner()
    args = [full[n] for n in in_names]
    zeros = [np.zeros((NCORES * a.shape[0], *a.shape[1:]), a.dtype)
             for a in out_avals]
    outs = sharded(*args, *zeros)
    return postprocess_output(outs[0])
